# revision 7
# baseline (speedup 1.0000x reference)
"""MllamaTextCrossAttention kernel for 8 Trainium2 NeuronCores.

Strategy: tensor-parallel over heads (4 q-heads + 1 kv-head per core).
Each core computes q/k/v projections for its shard, fused QK-RMS-norm,
block-diagonal varlen attention (segments baked in at build time from the
actual cu_seqlen values), and a row-parallel o_proj partial of the full
[1024, 4096] output. The host sums the 8 partials.

All heavy matmuls run in float32r (fp32 storage, ~4x faster PE mode).
Attention runs fully in transposed layout (features on partitions):
  qT [d, tok], kT [d, kpos], scoresT [kpos, q], attnT [d, q]
so no device transposes are needed except v (PE-transpose via identity).
Softmax denominators / RMS statistics use ones-matmul partition reductions;
per-q broadcasts use K=1 outer-product matmuls.
"""
import os
import sys

if "/opt/trn_rl_repo" not in sys.path:
    sys.path.insert(0, "/opt/trn_rl_repo")

import numpy as np

HIDDEN = 4096
N_HEADS = 32
N_KV = 8
HD = 128
EPS = 1e-5
SCALE = HD ** -0.5
TQ = 1024
TK = 6404
TKP = 6656          # TK padded to 13*512
KTILES = TKP // 128  # 52
SLABS = TKP // 512   # 13
NCORES = 8
HPC = N_HEADS // NCORES  # 4 q-heads per core
P = 128
NC = HIDDEN // P     # 32 contraction chunks


def _segments(cu_q, cu_k):
    eq = [0] + [int(min(max(int(v), 0), TQ)) for v in cu_q] + [TQ]
    ek = [0] + [int(min(max(int(v), 0), TK)) for v in cu_k] + [TK]
    segs = []
    for i in range(len(eq) - 1):
        q0, q1 = eq[i], eq[i + 1]
        k0, k1 = ek[i], ek[i + 1]
        if q1 <= q0:
            continue
        if k1 <= k0:
            segs.append((q0, q1, 0, TK, True))   # empty kv -> uniform over all Tk
        else:
            segs.append((q0, q1, k0, k1, False))
    return segs


def _build(segs):
    import concourse.bass as bass
    import concourse.tile as tile
    from concourse import bacc, mybir

    F32 = mybir.dt.float32
    F32R = mybir.dt.float32r
    AF = mybir.ActivationFunctionType
    MUL = mybir.AluOpType.mult

    nc = bacc.Bacc("TRN2", target_bir_lowering=False, debug=False,
                   num_devices=NCORES)

    hT = nc.declare_dram_parameter("hT", [HIDDEN, TQ], F32R, isOutput=False)
    cT = nc.declare_dram_parameter("cT", [HIDDEN, TKP], F32R, isOutput=False)
    wqT = nc.declare_dram_parameter("wqT", [HIDDEN, P * HPC], F32R, isOutput=False)
    wkT = nc.declare_dram_parameter("wkT", [HIDDEN, P], F32R, isOutput=False)
    wvT = nc.declare_dram_parameter("wvT", [HIDDEN, P], F32R, isOutput=False)
    woT = nc.declare_dram_parameter("woT", [P * HPC, HIDDEN], F32R, isOutput=False)
    wqk = nc.declare_dram_parameter("wqk", [1, P], F32, isOutput=False)
    onec = nc.declare_dram_parameter("onec", [P, 1], F32R, isOutput=False)
    onec2 = nc.declare_dram_parameter("onec2", [P, 2], F32R, isOutput=False)
    oner = nc.declare_dram_parameter("oner", [1, P], F32, isOutput=False)
    identd = nc.declare_dram_parameter("identd", [P, P], F32R, isOutput=False)
    zerosd = nc.declare_dram_parameter("zerosd", [P, 512], F32R, isOutput=False)
    onesd = nc.declare_dram_parameter("onesd", [P, 512], F32R, isOutput=False)
    out = nc.declare_dram_parameter("o", [TQ, HIDDEN], F32, isOutput=True)

    with tile.TileContext(nc) as tc:
        with tc.tile_pool(name="persist", bufs=1) as pp:
            qT = pp.tile([P, HPC, TQ + 2], F32R)      # q transposed (+2 pad cols)
            kT = pp.tile([P, KTILES, P], F32R)        # k transposed, 128-blocks
            vN = pp.tile([P, KTILES, P], F32R)        # v natural, 128-blocks
            aT = pp.tile([P, HPC, TQ], F32R)          # attn output transposed
            A_sb = pp.tile([P, KTILES], F32)          # 1/rms_k per kpos
            onec_sb = pp.tile([P, 1], F32R)
            onec2_sb = pp.tile([P, 2], F32R)
            oner_sb = pp.tile([1, P], F32)
            wqk_sb = pp.tile([1, P], F32)
            ident_sb = pp.tile([P, P], F32R)
            eps_sb = pp.tile([P, 1], F32)
            nc.vector.memset(eps_sb[:], EPS)
            nc.sync.dma_start(onec_sb[:], onec[:])
            nc.sync.dma_start(onec2_sb[:], onec2[:])
            nc.sync.dma_start(oner_sb[:], oner[:])
            nc.sync.dma_start(wqk_sb[:], wqk[:])
            nc.sync.dma_start(ident_sb[:], identd[:])

            # ---------------- Phase Q: q projection + RMS + scale ----------
            with tc.tile_pool(name="qp", bufs=3) as qp, \
                 tc.tile_pool(name="qps", bufs=1, space="PSUM") as qps, \
                 tc.tile_pool(name="qps2", bufs=2, space="PSUM") as qps2:
                for half in range(2):
                    tsl = slice(half * 512, (half + 1) * 512)
                    psq = [qps.tile([P, 512], F32, tag=f"q{f}", name=f"psq{f}")
                           for f in range(HPC)]
                    for c in range(NC):
                        ht_c = qp.tile([P, 512], F32R, tag="hTc")
                        nc.sync.dma_start(ht_c[:], hT[c * P:(c + 1) * P, tsl])
                        wq_c = qp.tile([P, P * HPC], F32R, tag="wqc")
                        nc.sync.dma_start(wq_c[:], wqT[c * P:(c + 1) * P, :])
                        for f in range(HPC):
                            nc.tensor.matmul(psq[f][:], wq_c[:, f * P:(f + 1) * P],
                                             ht_c[:], start=(c == 0), stop=(c == NC - 1))
                    for f in range(HPC):
                        qsq = qp.tile([P, 512], F32R, tag="qsq")
                        nc.scalar.copy(qT[:, f, tsl], psq[f][:])
                        nc.scalar.square(qsq[:], psq[f][:])
                        pss = qps2.tile([1, 512], F32, tag="pss")
                        nc.tensor.matmul(pss[:], onec_sb[:], qsq[:],
                                         start=True, stop=True)
                        sq = qp.tile([1, 512], F32, tag="sq")
                        nc.scalar.activation(sq[:], pss[:], AF.Sqrt,
                                             bias=eps_sb[0:1], scale=1.0 / HD)
                        bvec = qp.tile([1, 512], F32, tag="bvec")
                        nc.vector.reciprocal(bvec[:], sq[:])
                        psb = qps2.tile([P, 512], F32, tag="psb")
                        nc.tensor.matmul(psb[:], wqk_sb[:], bvec[:],
                                         start=True, stop=True)
                        nc.vector.tensor_tensor(qT[:, f, tsl], qT[:, f, tsl],
                                                psb[:], MUL)

            # ---------------- Phase KV: k/v projections + k-RMS + v-transpose
            with tc.tile_pool(name="kvw", bufs=1) as kvw, \
                 tc.tile_pool(name="kvp", bufs=2) as kvp, \
                 tc.tile_pool(name="kvps", bufs=2, space="PSUM") as kvps, \
                 tc.tile_pool(name="kvps2", bufs=1, space="PSUM") as kvps2:
                wk_sb = kvw.tile([P, NC, P], F32R)
                wv_sb = kvw.tile([P, NC, P], F32R)
                nc.sync.dma_start(wk_sb[:], wkT.rearrange("(c p) f -> p c f", p=P))
                nc.sync.dma_start(wv_sb[:], wvT.rearrange("(c p) f -> p c f", p=P))
                cT3 = cT.rearrange("(c p) n -> p c n", p=P)
                for s in range(SLABS):
                    ksl = slice(s * 512, (s + 1) * 512)
                    psk = kvps.tile([P, 512], F32, tag="psk")
                    psv = kvps.tile([P, 512], F32, tag="psv")
                    for q4 in range(4):
                        ct_q = kvp.tile([P, 8, 512], F32R, tag="ctq")
                        nc.sync.dma_start(ct_q[:], cT3[:, q4 * 8:(q4 + 1) * 8, ksl])
                        for cc in range(8):
                            c = q4 * 8 + cc
                            nc.tensor.matmul(psk[:], wk_sb[:, c, :], ct_q[:, cc, :],
                                             start=(c == 0), stop=(c == NC - 1))
                            nc.tensor.matmul(psv[:], wv_sb[:, c, :], ct_q[:, cc, :],
                                             start=(c == 0), stop=(c == NC - 1))
                    nc.scalar.copy(kT[:, 4 * s:4 * s + 4, :], psk[:])
                    ksq = kvp.tile([P, 512], F32R, tag="ksq")
                    nc.scalar.square(ksq[:], psk[:])
                    vstage = kvp.tile([P, 512], F32R, tag="vstage")
                    nc.scalar.copy(vstage[:], psv[:])
                    for t in range(4):
                        psr = kvps2.tile([P, 2], F32, tag="psr")
                        nc.tensor.matmul(psr[:], ksq[:, t * P:(t + 1) * P],
                                         onec2_sb[:], start=True, stop=True)
                        sqk = kvp.tile([P, 1], F32, tag="sqk")
                        nc.scalar.activation(sqk[:], psr[:, 0:1], AF.Sqrt,
                                             bias=eps_sb[:], scale=1.0 / HD)
                        nc.vector.reciprocal(A_sb[:, 4 * s + t:4 * s + t + 1], sqk[:])
                        pst = kvps2.tile([P, P], F32R, tag="pst")
                        nc.tensor.transpose(pst[:], vstage[:, t * P:(t + 1) * P],
                                            ident_sb[:])
                        nc.vector.tensor_copy(vN[:, 4 * s + t, :], pst[:])

            # ---------------- Phase ATTN: per-segment block attention -------
            with tc.tile_pool(name="ap", bufs=2) as ap, \
                 tc.tile_pool(name="aps", bufs=2, space="PSUM") as aps, \
                 tc.tile_pool(name="aps2", bufs=1, space="PSUM") as aps2:
                for (q0, q1, k0, k1, special) in segs:
                    t0 = k0 // P
                    t1 = (k1 + P - 1) // P
                    nt = t1 - t0
                    for qc0 in range(q0, q1, 512):
                        qc1 = min(qc0 + 512, q1)
                        nq = qc1 - qc0
                        nqp = nq + (nq & 1)   # fp32r needs even free sizes
                        for h in range(HPC):
                            E = ap.tile([P, nt, nqp], F32R, tag="E")
                            for ti in range(nt):
                                t = t0 + ti
                                lo = max(k0, t * P) - t * P
                                hi = min(k1, (t + 1) * P) - t * P
                                if special:
                                    if lo > 0:
                                        nc.sync.dma_start(E[0:lo, ti, :],
                                                          zerosd[0:lo, :nqp])
                                    nc.sync.dma_start(E[lo:hi, ti, :],
                                                      onesd[lo:hi, :nqp])
                                    if hi < P:
                                        nc.sync.dma_start(E[hi:P, ti, :],
                                                          zerosd[hi:P, :nqp])
                                    continue
                                pss = aps.tile([P, nqp], F32, tag="pss")
                                nc.tensor.matmul(pss[:], kT[:, t, :],
                                                 qT[:, h, qc0:qc0 + nqp],
                                                 start=True, stop=True)
                                # exp the full tile (ACT needs 32-aligned
                                # partition bases), then zero rows outside
                                # the segment's k-range
                                nc.scalar.activation(E[:, ti, :], pss[:],
                                                     AF.Exp,
                                                     scale=A_sb[:, t:t + 1])
                                if lo > 0:
                                    nc.sync.dma_start(E[0:lo, ti, :],
                                                      zerosd[0:lo, :nqp])
                                if hi < P:
                                    nc.sync.dma_start(E[hi:P, ti, :],
                                                      zerosd[hi:P, :nqp])
                            psd = aps2.tile([1, nqp], F32, tag="psd")
                            for ti in range(nt):
                                nc.tensor.matmul(psd[:], onec_sb[:], E[:, ti, :],
                                                 start=(ti == 0), stop=(ti == nt - 1))
                            rden = ap.tile([1, nqp], F32, tag="rden")
                            nc.vector.reciprocal(rden[:], psd[:])
                            psb = aps2.tile([P, nqp], F32, tag="psbA")
                            nc.tensor.matmul(psb[:], oner_sb[:], rden[:],
                                             start=True, stop=True)
                            bden = ap.tile([P, nqp], F32, tag="bden")
                            nc.scalar.copy(bden[:], psb[:])
                            psa = aps.tile([P, nqp], F32, tag="psa")
                            for ti in range(nt):
                                t = t0 + ti
                                nc.tensor.matmul(psa[:], vN[:, t, :], E[:, ti, :],
                                                 start=(ti == 0), stop=(ti == nt - 1))
                            nc.vector.tensor_tensor(aT[:, h, qc0:qc1],
                                                    psa[:, 0:nq],
                                                    bden[:, 0:nq], MUL)

            # ---------------- Phase O: o_proj partial ----------------------
            with tc.tile_pool(name="op", bufs=3) as op, \
                 tc.tile_pool(name="ops", bufs=4, space="PSUM") as ops:
                woT3 = woT.rearrange("(co p) n -> p co n", p=P)
                for n8 in range(8):
                    nsl = slice(n8 * 512, (n8 + 1) * 512)
                    wo_t = op.tile([P, HPC, 512], F32R, tag="wot")
                    nc.sync.dma_start(wo_t[:], woT3[:, :, nsl])
                    for qt in range(8):
                        pso = ops.tile([P, 512], F32, tag="pso")
                        for co in range(HPC):
                            nc.tensor.matmul(pso[:], aT[:, co, qt * P:(qt + 1) * P],
                                             wo_t[:, co, :],
                                             start=(co == 0), stop=(co == HPC - 1))
                        osb = op.tile([P, 512], F32, tag="osb")
                        nc.scalar.copy(osb[:], pso[:])
                        nc.sync.dma_start(out[qt * P:(qt + 1) * P, nsl], osb[:])

    nc.finalize()
    return nc


def _prepare(inputs):
    gi = {k: np.asarray(v) for k, v in inputs.items()}
    hs = np.ascontiguousarray(gi["hidden_states"], dtype=np.float32)
    cs = np.ascontiguousarray(gi["cross_attention_states"], dtype=np.float32)
    Wq = np.ascontiguousarray(gi["Wq"], dtype=np.float32)
    Wk = np.ascontiguousarray(gi["Wk"], dtype=np.float32)
    Wv = np.ascontiguousarray(gi["Wv"], dtype=np.float32)
    Wo = np.ascontiguousarray(gi["Wo"], dtype=np.float32)
    qw = np.asarray(gi["q_norm_w"], dtype=np.float32).reshape(-1)
    kw = np.asarray(gi["k_norm_w"], dtype=np.float32).reshape(-1)
    cu_q = np.asarray(gi["cu_seqlen_q"]).reshape(-1)
    cu_k = np.asarray(gi["cu_seqlen_k"]).reshape(-1)

    segs = _segments(cu_q, cu_k)
    nc = _build(segs)

    hT = np.ascontiguousarray(hs.T)                      # [4096, 1024]
    cTp = np.zeros((HIDDEN, TKP), np.float32)
    cTp[:, :TK] = cs.T
    wqkv = (qw * kw * SCALE).reshape(1, P).astype(np.float32)
    onec = np.ones((P, 1), np.float32)
    onec2 = np.ones((P, 2), np.float32)
    oner = np.ones((1, P), np.float32)
    ident = np.eye(P, dtype=np.float32)
    zeros = np.zeros((P, 512), np.float32)
    ones = np.ones((P, 512), np.float32)

    in_maps = []
    for c in range(NCORES):
        fsl = slice(c * P * HPC, (c + 1) * P * HPC)
        ksl = slice(c * P, (c + 1) * P)
        in_maps.append({
            "hT": hT,
            "cT": cTp,
            "wqT": np.ascontiguousarray(Wq[fsl, :].T),
            "wkT": np.ascontiguousarray(Wk[ksl, :].T),
            "wvT": np.ascontiguousarray(Wv[ksl, :].T),
            "woT": np.ascontiguousarray(Wo[:, fsl].T),
            "wqk": wqkv,
            "onec": onec,
            "onec2": onec2,
            "oner": oner,
            "identd": ident,
            "zerosd": zeros,
            "onesd": ones,
        })

    return nc, in_maps


def _reduce(results) -> np.ndarray:
    o = np.zeros((TQ, HIDDEN), np.float64)
    for c in range(NCORES):
        o += results[c]["o"].astype(np.float64)
    return o.astype(np.float32)


def kernel(**inputs) -> np.ndarray:
    from concourse.bass_utils import run_bass_kernel_spmd

    nc, in_maps = _prepare(inputs)
    r = run_bass_kernel_spmd(nc, in_maps, list(range(NCORES)))
    return _reduce(r.results)
